# revision 19
# baseline (speedup 1.0000x reference)
"""Multi-head causal attention (B=2, S=2048, D=1024, H=16) on 8 trn2 NeuronCores.

Sharding: core c handles batch b = c//4 and head group g = c%4 (heads 4g..4g+3).
Each core computes qkv projection, causal attention (scoresT layout) and the
partial output projection for its 4 heads; the host sums the 4 partials per
batch.

Schedule (v4), designed around measured engine costs (matmul ~N/2.4GHz
back-to-back, ACTIVATE ~(N+352)/1.2ns, DVE ~250ns/op overhead, HAM power
throttle under sustained PE activity):
  - Scalar engine runs ONLY exp: one strided [128, 2, 512-lo] ACTIVATE per
    (head-pair, key-tile) out of a 2-bank PSUM scores group holding both
    heads of the pair side by side.
  - Scores matmuls of a head pair go to PE row groups 0-63/64-127 and are
    emitted back-to-back so they run CONCURRENTLY on the split array (K=64).
  - Input DMAs are split across both HWDGE queues (SP=weights, ACT=x).
  - ctx accumulates per head in [65,512] PSUM tiles; softmax denominators are
    copied to SBUF, reciprocal'd with reciprocal_approx_fast, broadcast once
    per pair on gpsimd ([64,1024]), applied on DVE.
  - qkv projection of block nb+1 / out-projection of block qb-1 interleave
    into attention(qb) chunks (one fill per chunk, adaptively more when
    backlogged); the final out-projection double-buffers through the (by
    then free) scores PSUM pool with evacuations split scalar/vector.
  - 1/sqrt(HD) is folded into wq on the host.
"""

import sys
from contextlib import ExitStack

for _p in ("/opt/trn_rl_repo",):
    if _p not in sys.path:
        sys.path.insert(0, _p)

import numpy as np

import concourse.bass as bass  # noqa: F401
import concourse.tile as tile
from concourse import bacc, bass_utils, mybir

B, S, D, H, HD = 2, 2048, 1024, 16, 64
P = 128
NCORES = 8
NT = S // P          # 16 token tiles
KD = D // P          # 8 contraction tiles over D
NB = S // 512        # 4 query blocks of 512
HPC = 4              # heads per core
WCOLS = HPC * HD     # 256 weight columns per core per q/k/v

F32 = mybir.dt.float32
BF16 = mybir.dt.bfloat16
EXP = mybir.ActivationFunctionType.Exp

DT = BF16


def prep(x: np.ndarray) -> np.ndarray:
    import ml_dtypes

    return np.ascontiguousarray(x, np.float32).astype(ml_dtypes.bfloat16)


def _emit(tc: tile.TileContext, aps: dict):
    nc = tc.nc
    xT, wq, wk, wv, wo, tri, out = (
        aps["xT"], aps["wq"], aps["wk"], aps["wv"], aps["wo"],
        aps["tri"], aps["out"],
    )

    with ExitStack() as top:
        qk_pool = top.enter_context(tc.tile_pool(name="qk", bufs=4))
        v_pool = top.enter_context(tc.tile_pool(name="v1", bufs=NT))
        ctx_pool = top.enter_context(tc.tile_pool(name="ctxT", bufs=2))
        wo_pool = top.enter_context(tc.tile_pool(name="wo", bufs=2))
        const_pool = top.enter_context(tc.tile_pool(name="const", bufs=1))
        small_pool = top.enter_context(tc.tile_pool(name="small", bufs=4))
        out_pool = top.enter_context(tc.tile_pool(name="outsb", bufs=3))
        exp_pool = top.enter_context(tc.tile_pool(name="expT", bufs=5))
        x_pool = top.enter_context(tc.tile_pool(name="xc", bufs=NB))
        w_pool = top.enter_context(tc.tile_pool(name="w", bufs=3))
        # PSUM: sc 2x[128,1024] (banks 0-3), ctx 2x[65,512] (banks 4-5),
        # pp 2x[128,512] shared by qkv-proj + out-proj fills (banks 6-7)
        sc_pool = top.enter_context(tc.tile_pool(name="sc", bufs=2, space="PSUM"))
        ctxps_pool = top.enter_context(
            tc.tile_pool(name="ctxps", bufs=2, space="PSUM")
        )
        pp_pool = top.enter_context(tc.tile_pool(name="pp", bufs=2, space="PSUM"))

        # persistent SBUF tiles
        qT = [qk_pool.tile([P, S], DT, tag="qk", name=f"qT{i}") for i in range(2)]
        kT = [qk_pool.tile([P, S], DT, tag="qk", name=f"kT{i}") for i in range(2)]
        v1 = [
            v_pool.tile([P, HPC * (HD + 1)], DT, tag="v1", name=f"v1_{i}")
            for i in range(NT)
        ]
        ctxT = [
            ctx_pool.tile([P, S], DT, tag="ctxT", name=f"ctxT{i}")
            for i in range(2)
        ]
        wo_sb = [wo_pool.tile([P, D], DT, tag="wo", name=f"wo{i}") for i in range(2)]
        tri_sb = const_pool.tile([P, P], DT, tag="tri")

        w_all = {
            n: w_pool.tile([P, KD * WCOLS], DT, tag="w", name=f"w_{n}")
            for n in ("q", "k", "v")
        }
        wq_sb = [w_all["q"][:, kt * WCOLS : (kt + 1) * WCOLS] for kt in range(KD)]
        wk_sb = [w_all["k"][:, kt * WCOLS : (kt + 1) * WCOLS] for kt in range(KD)]
        wv_sb = [w_all["v"][:, kt * WCOLS : (kt + 1) * WCOLS] for kt in range(KD)]
        xc_all = [
            x_pool.tile([P, KD * 512], DT, tag="xc", name=f"xca{nb}")
            for nb in range(NB)
        ]
        xc = {
            (kt, nb): xc_all[nb][:, kt * 512 : (kt + 1) * 512]
            for kt in range(KD)
            for nb in range(NB)
        }

        # DMA: one batched strided transfer per weight matrix / x block —
        # queue-instruction count (not bandwidth) gates the prologue.
        # Weights on the SP queue, x blocks on the ACT queue.
        for n, src in (("q", wq), ("k", wk), ("v", wv)):
            nc.sync.dma_start(w_all[n][:], src[:])
        nc.sync.dma_start(tri_sb[:], tri[:])
        for nb in range(NB):
            nc.scalar.dma_start(
                xc_all[nb][:], xT[:, nb * KD * 512 : (nb + 1) * KD * 512]
            )
        for i in range(2):
            nc.sync.dma_start(wo_sb[i][:], wo[i * P : (i + 1) * P, :])

        # ones column of v1: memset whole tile once, value region is
        # overwritten by the v-projection evacuations afterwards
        for tt in range(NT):
            nc.vector.memset(v1[tt][:], 1.0)

        # ---- qkv projection fills (one closure per PSUM fill) --------------
        def proj_fills(nb):
            fills = []

            def qk_fill(w_sb, dstT, p, nb=nb):
                def go():
                    ps = pp_pool.tile([P, 512], F32, tag="pp")
                    for kt in range(KD):
                        nc.tensor.matmul(
                            ps[:],
                            w_sb[kt][:, p * P : (p + 1) * P],
                            xc[(kt, nb)][:],
                            start=(kt == 0),
                            stop=(kt == KD - 1),
                        )
                    nc.vector.tensor_copy(
                        dstT[p][:, nb * 512 : (nb + 1) * 512], ps[:]
                    )

                return go

            def v_fill(tloc, nb=nb):
                def go():
                    tt = nb * 4 + tloc
                    ps = pp_pool.tile([P, 512], F32, tag="pp")
                    for kt in range(KD):
                        nc.tensor.matmul(
                            ps[:, 0:WCOLS],
                            xc[(kt, nb)][:, tloc * P : (tloc + 1) * P],
                            wv_sb[kt][:],
                            start=(kt == 0),
                            stop=(kt == KD - 1),
                        )
                    v1_view = v1[tt][:].rearrange("p (a c) -> p a c", c=HD + 1)
                    nc.vector.tensor_copy(
                        v1_view[:, :, 0:HD],
                        ps[:, 0:WCOLS].rearrange("p (a c) -> p a c", c=HD),
                    )

                return go

            # pair-0 work first so attention(nb) can start after 6 fills
            fills.append(qk_fill(wq_sb, qT, 0))
            fills.append(qk_fill(wk_sb, kT, 0))
            for tloc in range(4):
                fills.append(v_fill(tloc))
            fills.append(qk_fill(wq_sb, qT, 1))
            fills.append(qk_fill(wk_sb, kT, 1))
            return fills

        # ---- out-projection fills for query block qb -----------------------
        def outproj_fills(qb, final=False):
            fills = []

            def o_fill(tt, i):
                def go():
                    if final:
                        pso = sc_pool.tile([P, 1024], F32, tag="sc")
                        osb = out_pool.tile([P, 1024], DT, tag="osb")
                        # per-half: 2 mms, then evac (scalar/vector alternate)
                        # and DMA immediately so the tail pipelines
                        for ob in range(2):
                            for kt2 in range(2):
                                nc.tensor.matmul(
                                    pso[:, ob * 512 : ob * 512 + 512],
                                    ctxT[kt2][:, tt * P : (tt + 1) * P],
                                    wo_sb[kt2][:, ob * 512 : (ob + 1) * 512],
                                    start=(kt2 == 0),
                                    stop=(kt2 == 1),
                                )
                            half = slice(ob * 512, ob * 512 + 512)
                            if ob == 0:
                                nc.scalar.copy(osb[:, half], pso[:, half])
                                nc.scalar.dma_start(
                                    out[tt * P : (tt + 1) * P, half], osb[:, half]
                                )
                            else:
                                nc.vector.tensor_copy(osb[:, half], pso[:, half])
                                nc.sync.dma_start(
                                    out[tt * P : (tt + 1) * P, half], osb[:, half]
                                )
                    else:
                        osb = out_pool.tile([P, 1024], DT, tag="osb")
                        for ob in range(2):
                            pso = pp_pool.tile([P, 512], F32, tag="pp")
                            for kt2 in range(2):
                                nc.tensor.matmul(
                                    pso[:],
                                    ctxT[kt2][:, tt * P : (tt + 1) * P],
                                    wo_sb[kt2][:, ob * 512 : (ob + 1) * 512],
                                    start=(kt2 == 0),
                                    stop=(kt2 == 1),
                                )
                            nc.vector.tensor_copy(
                                osb[:, ob * 512 : ob * 512 + 512], pso[:]
                            )
                        nc.sync.dma_start(out[tt * P : (tt + 1) * P, :], osb[:])

                return go

            for i, tloc in enumerate(range(4)):
                fills.append(o_fill(qb * 4 + tloc, i))
            return fills

        # ---- attention for query block qb, with interleaved fills ----------
        def emit_attention(qb, fills):
            q0 = qb * 512
            njt = 4 * qb + 4
            nchunks = 2 * ((njt + 1) // 2)
            chunk_i = 0
            for pair in range(2):
                p = pair
                ctxA = ctxps_pool.tile([65, 512], F32, tag="ctxps")
                ctxB = ctxps_pool.tile([65, 512], F32, tag="ctxps")
                for jt0 in range(0, njt, 2):
                    jts = [jt0] if jt0 + 1 >= njt else [jt0, jt0 + 1]
                    ets = []
                    # scores for the chunk (row-group pairs, 64x128 PE mode)
                    for jt in jts:
                        m = jt - 4 * qb
                        lo = P * m if m > 0 else 0
                        scp = sc_pool.tile([P, 1024], F32, tag="sc")
                        for off in (0, 64):
                            nc.tensor.matmul(
                                scp[:, (off // 64) * 512 + lo : (off // 64) * 512 + 512],
                                kT[p][off : off + 64, jt * P : (jt + 1) * P],
                                qT[p][off : off + 64, q0 + lo : q0 + 512],
                                start=True,
                                stop=True,
                            )
                        et = exp_pool.tile([P, 1024], DT, tag="expT")
                        # one exp over both heads, skipping the masked hole
                        scv = scp[:].rearrange("p (two n) -> p two n", two=2)
                        etv = et[:].rearrange("p (two n) -> p two n", two=2)
                        nc.scalar.activation(
                            etv[:, :, lo:512], scv[:, :, lo:512], EXP
                        )
                        ets.append((et, lo, jt, m))
                    # interleaved fill work keeps the PE busy during exp; its
                    # DVE evacuation is emitted BEFORE the exp-blocked tri
                    # masks so the next fill's matmuls aren't held up
                    chunk_i += 1
                    take = 0
                    if fills:
                        left = nchunks - chunk_i + 1
                        take = max(1, -(-len(fills) // max(1, left)))
                    for _ in range(min(take, len(fills))):
                        fills.pop(0)()
                    for et, lo, jt, m in ets:
                        if m >= 0:  # diagonal block: triangle mask multiply
                            for half in range(2):
                                h0 = half * 512
                                nc.vector.tensor_mul(
                                    et[:, h0 + lo : h0 + lo + P],
                                    et[:, h0 + lo : h0 + lo + P],
                                    tri_sb[:],
                                )
                    # attn @ v for the chunk (128x128 PE mode)
                    for et, lo, jt, m in ets:
                        for half, ctx in ((0, ctxA), (1, ctxB)):
                            h = 2 * pair + half
                            nc.tensor.matmul(
                                ctx[:, lo:512],
                                v1[jt][:, h * 65 : (h + 1) * 65],
                                et[:, half * 512 + lo : half * 512 + 512],
                                start=(jt == 0),
                                stop=(jt == njt - 1),
                                skip_group_check=True,
                            )
                # normalize: copy denominators to SBUF, reciprocal, broadcast
                # once per pair, scale into ctxT
                rec = small_pool.tile([1, 1024], F32, tag="rec")
                nc.vector.tensor_copy(rec[:, 0:512], ctxA[64:65, :])
                nc.vector.tensor_copy(rec[:, 512:1024], ctxB[64:65, :])
                reci = small_pool.tile([1, 1024], F32, tag="reci")
                nc.vector.reciprocal_approx_fast(reci[:], rec[:])
                recb = small_pool.tile([64, 1024], F32, tag="recb")
                nc.gpsimd.partition_broadcast(recb[:], reci[:], channels=64)
                for half, ctx in ((0, ctxA), (1, ctxB)):
                    nc.vector.tensor_mul(
                        ctxT[p][64 * half : 64 * half + 64, q0 : q0 + 512],
                        ctx[0:64, :],
                        recb[:, half * 512 : half * 512 + 512],
                    )

        # ===== schedule ======================================================
        fills0 = proj_fills(0)
        for f in fills0[:6]:  # pair-0 proj of block 0 inline
            f()
        carry = fills0[6:]
        for qb in range(NB):
            fills = carry
            carry = []
            if qb + 1 < NB:
                fills += proj_fills(qb + 1)
            if qb == 2:
                fills += outproj_fills(0)
            if qb == 3:
                fills += outproj_fills(1) + outproj_fills(2)
            emit_attention(qb, fills)
            for f in fills:  # anything not consumed by the chunk slots
                f()
        # final out-projection: double-buffer through the free scores pool,
        # half the evacuations on the (now idle) scalar engine
        for f in outproj_fills(NB - 1, final=True):
            f()


_BUILD_CACHE = {}


def build():
    if "nc" in _BUILD_CACHE:
        return _BUILD_CACHE["nc"]
    nc = bacc.Bacc("TRN2", target_bir_lowering=False, debug=False)
    aps = {
        "xT": nc.dram_tensor("xT", [P, NB * KD * 512], DT, kind="ExternalInput").ap(),
        "wq": nc.dram_tensor("wq", [P, KD * WCOLS], DT, kind="ExternalInput").ap(),
        "wk": nc.dram_tensor("wk", [P, KD * WCOLS], DT, kind="ExternalInput").ap(),
        "wv": nc.dram_tensor("wv", [P, KD * WCOLS], DT, kind="ExternalInput").ap(),
        "wo": nc.dram_tensor("wo", [WCOLS, D], DT, kind="ExternalInput").ap(),
        "tri": nc.dram_tensor("tri", [P, P], DT, kind="ExternalInput").ap(),
        "out": nc.dram_tensor("out", [S, D], DT, kind="ExternalOutput").ap(),
    }
    with tile.TileContext(nc) as tc:
        _emit(tc, aps)
    nc.compile()
    _BUILD_CACHE["nc"] = nc
    return nc


def make_tri() -> np.ndarray:
    """tri[dj, t] = 1 if dj <= t else 0 (causal keep within a 128 block)."""
    dj = np.arange(P)[:, None]
    t = np.arange(P)[None, :]
    return prep(np.where(dj <= t, 1.0, 0.0).astype(np.float32))


def swz_w(w):
    """[D, WCOLS] -> [P, KD*WCOLS]: SBUF-layout swizzle so the DMA is contiguous."""
    return np.ascontiguousarray(
        w.reshape(KD, P, WCOLS).transpose(1, 0, 2).reshape(P, KD * WCOLS)
    )


def swz_x(xb):
    """x[b] [S, D] -> xT swizzled [P, NB*KD*512] matching xc_all layout."""
    xT = xb.T  # [D, S]
    return np.ascontiguousarray(
        xT.reshape(KD, P, NB, 512).transpose(1, 2, 0, 3).reshape(P, NB * KD * 512)
    )


def make_in_maps(x, w_qkv, w_out):
    tri = make_tri()
    scale = 1.0 / np.sqrt(HD)
    in_maps = []
    for c in range(NCORES):
        b, g = c // 4, c % 4
        cs = slice(g * WCOLS, (g + 1) * WCOLS)
        in_maps.append(
            {
                "xT": prep(swz_x(x[b])),
                "wq": prep(swz_w(w_qkv[:, g * WCOLS : (g + 1) * WCOLS] * scale)),
                "wk": prep(swz_w(w_qkv[:, D + g * WCOLS : D + (g + 1) * WCOLS])),
                "wv": prep(swz_w(w_qkv[:, 2 * D + g * WCOLS : 2 * D + (g + 1) * WCOLS])),
                "wo": prep(w_out[cs, :]),
                "tri": tri,
            }
        )
    return in_maps


def kernel(x, w_qkv, w_out, _trace=False):
    nc = build()
    in_maps = make_in_maps(
        np.asarray(x, np.float32), np.asarray(w_qkv, np.float32),
        np.asarray(w_out, np.float32),
    )
    res = bass_utils.run_bass_kernel_spmd(
        nc, in_maps, core_ids=list(range(NCORES)), trace=_trace
    )
    outs = [np.asarray(res.results[c]["out"], np.float32) for c in range(NCORES)]
    full = np.stack(
        [sum(outs[b * 4 : (b + 1) * 4][1:], outs[b * 4]) for b in range(B)], axis=0
    )
    if _trace:
        kernel.last_results = res
    return full.astype(np.float32)


# revision 21
# speedup vs baseline: 1.0432x; 1.0432x over previous
"""Multi-head causal attention (B=2, S=2048, D=1024, H=16) on 8 trn2 NeuronCores.

Sharding: core c handles batch b = c//4 and head group g = c%4 (heads 4g..4g+3).
Each core computes qkv projection, causal attention (scoresT layout) and the
partial output projection for its 4 heads; the host sums the 4 partials per
batch.

Schedule (v4), designed around measured engine costs (matmul ~N/2.4GHz
back-to-back, ACTIVATE ~(N+352)/1.2ns, DVE ~250ns/op overhead, HAM power
throttle under sustained PE activity):
  - Scalar engine runs ONLY exp: one strided [128, 2, 512-lo] ACTIVATE per
    (head-pair, key-tile) out of a 2-bank PSUM scores group holding both
    heads of the pair side by side.
  - Scores matmuls of a head pair go to PE row groups 0-63/64-127 and are
    emitted back-to-back so they run CONCURRENTLY on the split array (K=64).
  - Input DMAs are split across both HWDGE queues (SP=weights, ACT=x).
  - ctx accumulates per head in [65,512] PSUM tiles; softmax denominators are
    copied to SBUF, reciprocal'd with reciprocal_approx_fast, broadcast once
    per pair on gpsimd ([64,1024]), applied on DVE.
  - qkv projection of block nb+1 / out-projection of block qb-1 interleave
    into attention(qb) chunks (one fill per chunk, adaptively more when
    backlogged); the final out-projection double-buffers through the (by
    then free) scores PSUM pool with evacuations split scalar/vector.
  - 1/sqrt(HD) is folded into wq on the host.
"""

import sys
from contextlib import ExitStack

for _p in ("/opt/trn_rl_repo",):
    if _p not in sys.path:
        sys.path.insert(0, _p)

import numpy as np

import concourse.bass as bass  # noqa: F401
import concourse.tile as tile
from concourse import bacc, bass_utils, mybir

B, S, D, H, HD = 2, 2048, 1024, 16, 64
P = 128
NCORES = 8
NT = S // P          # 16 token tiles
KD = D // P          # 8 contraction tiles over D
NB = S // 512        # 4 query blocks of 512
HPC = 4              # heads per core
WCOLS = HPC * HD     # 256 weight columns per core per q/k/v

F32 = mybir.dt.float32
BF16 = mybir.dt.bfloat16
EXP = mybir.ActivationFunctionType.Exp

DT = BF16


def prep(x: np.ndarray) -> np.ndarray:
    import ml_dtypes

    return np.ascontiguousarray(x, np.float32).astype(ml_dtypes.bfloat16)


def _emit(tc: tile.TileContext, aps: dict):
    nc = tc.nc
    xT, wq, wk, wv, wo, tri, out = (
        aps["xT"], aps["wq"], aps["wk"], aps["wv"], aps["wo"],
        aps["tri"], aps["out"],
    )

    with ExitStack() as top:
        qk_pool = top.enter_context(tc.tile_pool(name="qk", bufs=4))
        v_pool = top.enter_context(tc.tile_pool(name="v1", bufs=NT))
        ctx_pool = top.enter_context(tc.tile_pool(name="ctxT", bufs=2))
        wo_pool = top.enter_context(tc.tile_pool(name="wo", bufs=2))
        const_pool = top.enter_context(tc.tile_pool(name="const", bufs=1))
        small_pool = top.enter_context(tc.tile_pool(name="small", bufs=4))
        out_pool = top.enter_context(tc.tile_pool(name="outsb", bufs=3))
        exp_pool = top.enter_context(tc.tile_pool(name="expT", bufs=5))
        x_pool = top.enter_context(tc.tile_pool(name="xc", bufs=NB))
        w_pool = top.enter_context(tc.tile_pool(name="w", bufs=3))
        # PSUM: sc 2x[128,1024] (banks 0-3), ctx 2x[65,512] (banks 4-5),
        # pp 2x[128,512] shared by qkv-proj + out-proj fills (banks 6-7)
        sc_pool = top.enter_context(tc.tile_pool(name="sc", bufs=2, space="PSUM"))
        ctxps_pool = top.enter_context(
            tc.tile_pool(name="ctxps", bufs=2, space="PSUM")
        )
        pp_pool = top.enter_context(tc.tile_pool(name="pp", bufs=2, space="PSUM"))

        # persistent SBUF tiles
        qT = [qk_pool.tile([P, S], DT, tag="qk", name=f"qT{i}") for i in range(2)]
        kT = [qk_pool.tile([P, S], DT, tag="qk", name=f"kT{i}") for i in range(2)]
        v1 = [
            v_pool.tile([P, HPC * (HD + 1)], DT, tag="v1", name=f"v1_{i}")
            for i in range(NT)
        ]
        ctxT = [
            ctx_pool.tile([P, S], DT, tag="ctxT", name=f"ctxT{i}")
            for i in range(2)
        ]
        wo_sb = [wo_pool.tile([P, D], DT, tag="wo", name=f"wo{i}") for i in range(2)]
        tri_sb = const_pool.tile([P, P], DT, tag="tri")

        w_all = {
            n: w_pool.tile([P, KD * WCOLS], DT, tag="w", name=f"w_{n}")
            for n in ("q", "k", "v")
        }
        wq_sb = [w_all["q"][:, kt * WCOLS : (kt + 1) * WCOLS] for kt in range(KD)]
        wk_sb = [w_all["k"][:, kt * WCOLS : (kt + 1) * WCOLS] for kt in range(KD)]
        wv_sb = [w_all["v"][:, kt * WCOLS : (kt + 1) * WCOLS] for kt in range(KD)]
        xc_all = [
            x_pool.tile([P, KD * 512], DT, tag="xc", name=f"xca{nb}")
            for nb in range(NB)
        ]
        xc = {
            (kt, nb): xc_all[nb][:, kt * 512 : (kt + 1) * 512]
            for kt in range(KD)
            for nb in range(NB)
        }

        # DMA: one batched strided transfer per weight matrix / x block —
        # queue-instruction count (not bandwidth) gates the prologue.
        # Weights on the SP queue, x blocks on the ACT queue.
        for n, src in (("q", wq), ("k", wk), ("v", wv)):
            nc.sync.dma_start(w_all[n][:], src[:])
        nc.sync.dma_start(tri_sb[:], tri[:])
        for nb in range(NB):
            nc.scalar.dma_start(
                xc_all[nb][:], xT[:, nb * KD * 512 : (nb + 1) * KD * 512]
            )
        for i in range(2):
            nc.sync.dma_start(wo_sb[i][:], wo[i * P : (i + 1) * P, :])

        # ones column of v1: memset whole tile once, value region is
        # overwritten by the v-projection evacuations afterwards
        for tt in range(NT):
            nc.vector.memset(v1[tt][:], 1.0)

        # ---- qkv projection fills (one closure per PSUM fill) --------------
        def proj_fills(nb):
            fills = []

            def qk_fill(w_sb, dstT, p, nb=nb):
                def go():
                    ps = pp_pool.tile([P, 512], F32, tag="pp")
                    for kt in range(KD):
                        nc.tensor.matmul(
                            ps[:],
                            w_sb[kt][:, p * P : (p + 1) * P],
                            xc[(kt, nb)][:],
                            start=(kt == 0),
                            stop=(kt == KD - 1),
                        )
                    nc.vector.tensor_copy(
                        dstT[p][:, nb * 512 : (nb + 1) * 512], ps[:]
                    )

                return go

            def v_fill(tloc, nb=nb):
                def go():
                    tt = nb * 4 + tloc
                    ps = pp_pool.tile([P, 512], F32, tag="pp")
                    for kt in range(KD):
                        nc.tensor.matmul(
                            ps[:, 0:WCOLS],
                            xc[(kt, nb)][:, tloc * P : (tloc + 1) * P],
                            wv_sb[kt][:],
                            start=(kt == 0),
                            stop=(kt == KD - 1),
                        )
                    v1_view = v1[tt][:].rearrange("p (a c) -> p a c", c=HD + 1)
                    nc.vector.tensor_copy(
                        v1_view[:, :, 0:HD],
                        ps[:, 0:WCOLS].rearrange("p (a c) -> p a c", c=HD),
                    )

                return go

            # pair-0 work first so attention(nb) can start after 6 fills
            fills.append(qk_fill(wq_sb, qT, 0))
            fills.append(qk_fill(wk_sb, kT, 0))
            for tloc in range(4):
                fills.append(v_fill(tloc))
            fills.append(qk_fill(wq_sb, qT, 1))
            fills.append(qk_fill(wk_sb, kT, 1))
            return fills

        # ---- out-projection fills for query block qb -----------------------
        def outproj_fills(qb, final=False):
            fills = []

            def o_fill(tt, i):
                def go():
                    if final:
                        pso = sc_pool.tile([P, 1024], F32, tag="sc")
                        osb = out_pool.tile([P, 1024], DT, tag="osb")
                        # per-half: 2 mms, then evac (scalar/vector alternate)
                        # and DMA immediately so the tail pipelines
                        for ob in range(2):
                            for kt2 in range(2):
                                nc.tensor.matmul(
                                    pso[:, ob * 512 : ob * 512 + 512],
                                    ctxT[kt2][:, tt * P : (tt + 1) * P],
                                    wo_sb[kt2][:, ob * 512 : (ob + 1) * 512],
                                    start=(kt2 == 0),
                                    stop=(kt2 == 1),
                                )
                            half = slice(ob * 512, ob * 512 + 512)
                            if ob == 0:
                                nc.scalar.copy(osb[:, half], pso[:, half])
                                nc.scalar.dma_start(
                                    out[tt * P : (tt + 1) * P, half], osb[:, half]
                                )
                            else:
                                nc.vector.tensor_copy(osb[:, half], pso[:, half])
                                nc.sync.dma_start(
                                    out[tt * P : (tt + 1) * P, half], osb[:, half]
                                )
                    else:
                        osb = out_pool.tile([P, 1024], DT, tag="osb")
                        for ob in range(2):
                            pso = pp_pool.tile([P, 512], F32, tag="pp")
                            for kt2 in range(2):
                                nc.tensor.matmul(
                                    pso[:],
                                    ctxT[kt2][:, tt * P : (tt + 1) * P],
                                    wo_sb[kt2][:, ob * 512 : (ob + 1) * 512],
                                    start=(kt2 == 0),
                                    stop=(kt2 == 1),
                                )
                            nc.vector.tensor_copy(
                                osb[:, ob * 512 : ob * 512 + 512], pso[:]
                            )
                        nc.sync.dma_start(out[tt * P : (tt + 1) * P, :], osb[:])

                return go

            tlocs = [3] if final else range(4)
            for i, tloc in enumerate(tlocs):
                fills.append(o_fill(qb * 4 + tloc, i))
            return fills

        # ---- attention for query block qb, with interleaved fills ----------
        def emit_attention(qb, fills):
            q0 = qb * 512
            njt = 4 * qb + 4
            nchunks = 2 * ((njt + 1) // 2)
            chunk_i = 0
            for pair in range(2):
                p = pair
                ctxA = ctxps_pool.tile([65, 512], F32, tag="ctxps")
                ctxB = ctxps_pool.tile([65, 512], F32, tag="ctxps")
                for jt0 in range(0, njt, 2):
                    jts = [jt0] if jt0 + 1 >= njt else [jt0, jt0 + 1]
                    ets = []
                    # scores for the chunk (row-group pairs, 64x128 PE mode)
                    for jt in jts:
                        m = jt - 4 * qb
                        lo = P * m if m > 0 else 0
                        scp = sc_pool.tile([P, 1024], F32, tag="sc")
                        for off in (0, 64):
                            nc.tensor.matmul(
                                scp[:, (off // 64) * 512 + lo : (off // 64) * 512 + 512],
                                kT[p][off : off + 64, jt * P : (jt + 1) * P],
                                qT[p][off : off + 64, q0 + lo : q0 + 512],
                                start=True,
                                stop=True,
                            )
                        et = exp_pool.tile([P, 1024], DT, tag="expT")
                        # one exp over both heads, skipping the masked hole
                        scv = scp[:].rearrange("p (two n) -> p two n", two=2)
                        etv = et[:].rearrange("p (two n) -> p two n", two=2)
                        nc.scalar.activation(
                            etv[:, :, lo:512], scv[:, :, lo:512], EXP
                        )
                        if m >= 0:  # diagonal block: triangle mask multiply
                            for half in range(2):
                                h0 = half * 512
                                nc.vector.tensor_mul(
                                    et[:, h0 + lo : h0 + lo + P],
                                    et[:, h0 + lo : h0 + lo + P],
                                    tri_sb[:],
                                )
                        ets.append((et, lo, jt))
                    # interleaved fill work keeps the PE busy during exp
                    chunk_i += 1
                    take = 0
                    if fills:
                        left = nchunks - chunk_i + 1
                        take = max(1, -(-len(fills) // max(1, left)))
                    for _ in range(min(take, len(fills))):
                        fills.pop(0)()
                    # attn @ v for the chunk (128x128 PE mode)
                    for et, lo, jt in ets:
                        for half, ctx in ((0, ctxA), (1, ctxB)):
                            h = 2 * pair + half
                            nc.tensor.matmul(
                                ctx[:, lo:512],
                                v1[jt][:, h * 65 : (h + 1) * 65],
                                et[:, half * 512 + lo : half * 512 + 512],
                                start=(jt == 0),
                                stop=(jt == njt - 1),
                                skip_group_check=True,
                            )
                # normalize: copy denominators to SBUF, reciprocal, broadcast
                # once per pair, scale into ctxT
                rec = small_pool.tile([1, 1024], F32, tag="rec")
                nc.vector.tensor_copy(rec[:, 0:512], ctxA[64:65, :])
                nc.vector.tensor_copy(rec[:, 512:1024], ctxB[64:65, :])
                reci = small_pool.tile([1, 1024], F32, tag="reci")
                nc.vector.reciprocal_approx_fast(reci[:], rec[:])
                recb = small_pool.tile([64, 1024], F32, tag="recb")
                nc.gpsimd.partition_broadcast(recb[:], reci[:], channels=64)
                for half, ctx in ((0, ctxA), (1, ctxB)):
                    nc.vector.tensor_mul(
                        ctxT[p][64 * half : 64 * half + 64, q0 : q0 + 512],
                        ctx[0:64, :],
                        recb[:, half * 512 : half * 512 + 512],
                    )

        # ===== schedule ======================================================
        fills0 = proj_fills(0)
        for f in fills0[:6]:  # pair-0 proj of block 0 inline
            f()
        carry = fills0[6:]
        for qb in range(NB):
            fills = carry
            carry = []
            if qb + 1 < NB:
                fills += proj_fills(qb + 1)
            if qb == 2:
                fills += outproj_fills(0)
            if qb == 3:
                fills += outproj_fills(1) + outproj_fills(2)
            emit_attention(qb, fills)
            for f in fills:  # anything not consumed by the chunk slots
                f()
        # final out-projection. The kt2=0 half only needs pair-0's ctx
        # (ready before pair-1's normalize chain finishes), so pre-issue it
        # into held PSUM slots to keep the PE busy during that chain; then
        # finish kt2=1 + evac + DMA per token tile.
        held = []
        for tt in (12, 13):
            pso = sc_pool.tile([P, 1024], F32, tag="sc")
            for ob in range(2):
                nc.tensor.matmul(
                    pso[:, ob * 512 : ob * 512 + 512],
                    ctxT[0][:, tt * P : (tt + 1) * P],
                    wo_sb[0][:, ob * 512 : (ob + 1) * 512],
                    start=True,
                    stop=False,
                    skip_group_check=True,
                )
            held.append((tt, pso, 1024))
        for ob in range(2):
            pp = pp_pool.tile([P, 512], F32, tag="pp")
            nc.tensor.matmul(
                pp[:],
                ctxT[0][:, 14 * P : 15 * P],
                wo_sb[0][:, ob * 512 : (ob + 1) * 512],
                start=True,
                stop=False,
                skip_group_check=True,
            )
            held.append((14, pp, ob))
        for tt, pso, kind in held:
            if kind == 1024:
                osb = out_pool.tile([P, 1024], DT, tag="osb")
                for ob in range(2):
                    nc.tensor.matmul(
                        pso[:, ob * 512 : ob * 512 + 512],
                        ctxT[1][:, tt * P : (tt + 1) * P],
                        wo_sb[1][:, ob * 512 : (ob + 1) * 512],
                        start=False,
                        stop=True,
                        skip_group_check=True,
                    )
                    half = slice(ob * 512, ob * 512 + 512)
                    if ob == 0:
                        nc.scalar.copy(osb[:, half], pso[:, half])
                        nc.scalar.dma_start(
                            out[tt * P : (tt + 1) * P, half], osb[:, half]
                        )
                    else:
                        nc.vector.tensor_copy(osb[:, half], pso[:, half])
                        nc.sync.dma_start(
                            out[tt * P : (tt + 1) * P, half], osb[:, half]
                        )
            else:
                ob = kind
                half = slice(ob * 512, ob * 512 + 512)
                nc.tensor.matmul(
                    pso[:],
                    ctxT[1][:, tt * P : (tt + 1) * P],
                    wo_sb[1][:, ob * 512 : (ob + 1) * 512],
                    start=False,
                    stop=True,
                    skip_group_check=True,
                )
                osb = out_pool.tile([P, 512], DT, tag="osbh")
                if ob == 0:
                    nc.scalar.copy(osb[:], pso[:])
                    nc.scalar.dma_start(out[tt * P : (tt + 1) * P, half], osb[:])
                else:
                    nc.vector.tensor_copy(osb[:], pso[:])
                    nc.sync.dma_start(out[tt * P : (tt + 1) * P, half], osb[:])
        for f in outproj_fills(NB - 1, final=True):
            f()


_BUILD_CACHE = {}


def build():
    if "nc" in _BUILD_CACHE:
        return _BUILD_CACHE["nc"]
    nc = bacc.Bacc("TRN2", target_bir_lowering=False, debug=False)
    aps = {
        "xT": nc.dram_tensor("xT", [P, NB * KD * 512], DT, kind="ExternalInput").ap(),
        "wq": nc.dram_tensor("wq", [P, KD * WCOLS], DT, kind="ExternalInput").ap(),
        "wk": nc.dram_tensor("wk", [P, KD * WCOLS], DT, kind="ExternalInput").ap(),
        "wv": nc.dram_tensor("wv", [P, KD * WCOLS], DT, kind="ExternalInput").ap(),
        "wo": nc.dram_tensor("wo", [WCOLS, D], DT, kind="ExternalInput").ap(),
        "tri": nc.dram_tensor("tri", [P, P], DT, kind="ExternalInput").ap(),
        "out": nc.dram_tensor("out", [S, D], DT, kind="ExternalOutput").ap(),
    }
    with tile.TileContext(nc) as tc:
        _emit(tc, aps)
    nc.compile()
    _BUILD_CACHE["nc"] = nc
    return nc


def make_tri() -> np.ndarray:
    """tri[dj, t] = 1 if dj <= t else 0 (causal keep within a 128 block)."""
    dj = np.arange(P)[:, None]
    t = np.arange(P)[None, :]
    return prep(np.where(dj <= t, 1.0, 0.0).astype(np.float32))


def swz_w(w):
    """[D, WCOLS] -> [P, KD*WCOLS]: SBUF-layout swizzle so the DMA is contiguous."""
    return np.ascontiguousarray(
        w.reshape(KD, P, WCOLS).transpose(1, 0, 2).reshape(P, KD * WCOLS)
    )


def swz_x(xb):
    """x[b] [S, D] -> xT swizzled [P, NB*KD*512] matching xc_all layout."""
    xT = xb.T  # [D, S]
    return np.ascontiguousarray(
        xT.reshape(KD, P, NB, 512).transpose(1, 2, 0, 3).reshape(P, NB * KD * 512)
    )


def make_in_maps(x, w_qkv, w_out):
    tri = make_tri()
    scale = 1.0 / np.sqrt(HD)
    in_maps = []
    for c in range(NCORES):
        b, g = c // 4, c % 4
        cs = slice(g * WCOLS, (g + 1) * WCOLS)
        in_maps.append(
            {
                "xT": prep(swz_x(x[b])),
                "wq": prep(swz_w(w_qkv[:, g * WCOLS : (g + 1) * WCOLS] * scale)),
                "wk": prep(swz_w(w_qkv[:, D + g * WCOLS : D + (g + 1) * WCOLS])),
                "wv": prep(swz_w(w_qkv[:, 2 * D + g * WCOLS : 2 * D + (g + 1) * WCOLS])),
                "wo": prep(w_out[cs, :]),
                "tri": tri,
            }
        )
    return in_maps


def kernel(x, w_qkv, w_out, _trace=False):
    nc = build()
    in_maps = make_in_maps(
        np.asarray(x, np.float32), np.asarray(w_qkv, np.float32),
        np.asarray(w_out, np.float32),
    )
    res = bass_utils.run_bass_kernel_spmd(
        nc, in_maps, core_ids=list(range(NCORES)), trace=_trace
    )
    outs = [np.asarray(res.results[c]["out"], np.float32) for c in range(NCORES)]
    full = np.stack(
        [sum(outs[b * 4 : (b + 1) * 4][1:], outs[b * 4]) for b in range(B)], axis=0
    )
    if _trace:
        kernel.last_results = res
    return full.astype(np.float32)


# revision 23
# speedup vs baseline: 1.0684x; 1.0242x over previous
"""Multi-head causal attention (B=2, S=2048, D=1024, H=16) on 8 trn2 NeuronCores.

Sharding: core c handles batch b = c//4 and head group g = c%4 (heads 4g..4g+3).
Each core computes qkv projection, causal attention (scoresT layout) and the
partial output projection for its 4 heads; the host sums the 4 partials per
batch.

Schedule (v4), designed around measured engine costs (matmul ~N/2.4GHz
back-to-back, ACTIVATE ~(N+352)/1.2ns, DVE ~250ns/op overhead, HAM power
throttle under sustained PE activity):
  - Scalar engine runs ONLY exp: one strided [128, 2, 512-lo] ACTIVATE per
    (head-pair, key-tile) out of a 2-bank PSUM scores group holding both
    heads of the pair side by side.
  - Scores matmuls of a head pair go to PE row groups 0-63/64-127 and are
    emitted back-to-back so they run CONCURRENTLY on the split array (K=64).
  - Input DMAs are split across both HWDGE queues (SP=weights, ACT=x).
  - ctx accumulates per head in [65,512] PSUM tiles; softmax denominators are
    copied to SBUF, reciprocal'd with reciprocal_approx_fast, broadcast once
    per pair on gpsimd ([64,1024]), applied on DVE.
  - qkv projection of block nb+1 / out-projection of block qb-1 interleave
    into attention(qb) chunks (one fill per chunk, adaptively more when
    backlogged); the final out-projection double-buffers through the (by
    then free) scores PSUM pool with evacuations split scalar/vector.
  - 1/sqrt(HD) is folded into wq on the host.
"""

import sys
from contextlib import ExitStack

for _p in ("/opt/trn_rl_repo",):
    if _p not in sys.path:
        sys.path.insert(0, _p)

import numpy as np

import concourse.bass as bass  # noqa: F401
import concourse.tile as tile
from concourse import bacc, bass_utils, mybir

B, S, D, H, HD = 2, 2048, 1024, 16, 64
P = 128
NCORES = 8
NT = S // P          # 16 token tiles
KD = D // P          # 8 contraction tiles over D
NB = S // 512        # 4 query blocks of 512
HPC = 4              # heads per core
WCOLS = HPC * HD     # 256 weight columns per core per q/k/v

F32 = mybir.dt.float32
BF16 = mybir.dt.bfloat16
EXP = mybir.ActivationFunctionType.Exp

DT = BF16


def prep(x: np.ndarray) -> np.ndarray:
    import ml_dtypes

    return np.ascontiguousarray(x, np.float32).astype(ml_dtypes.bfloat16)


def _emit(tc: tile.TileContext, aps: dict):
    nc = tc.nc
    xT, wq, wk, wv, wo, tri, out = (
        aps["xT"], aps["wq"], aps["wk"], aps["wv"], aps["wo"],
        aps["tri"], aps["out"],
    )

    with ExitStack() as top:
        qk_pool = top.enter_context(tc.tile_pool(name="qk", bufs=4))
        v_pool = top.enter_context(tc.tile_pool(name="v1", bufs=NT))
        ctx_pool = top.enter_context(tc.tile_pool(name="ctxT", bufs=2))
        wo_pool = top.enter_context(tc.tile_pool(name="wo", bufs=2))
        const_pool = top.enter_context(tc.tile_pool(name="const", bufs=1))
        small_pool = top.enter_context(tc.tile_pool(name="small", bufs=4))
        out_pool = top.enter_context(tc.tile_pool(name="outsb", bufs=3))
        exp_pool = top.enter_context(tc.tile_pool(name="expT", bufs=5))
        x_pool = top.enter_context(tc.tile_pool(name="xc", bufs=NB))
        w_pool = top.enter_context(tc.tile_pool(name="w", bufs=3))
        # PSUM: sc 2x[128,1024] (banks 0-3), ctx 2x[65,512] (banks 4-5),
        # pp 2x[128,512] shared by qkv-proj + out-proj fills (banks 6-7)
        sc_pool = top.enter_context(tc.tile_pool(name="sc", bufs=2, space="PSUM"))
        ctxps_pool = top.enter_context(
            tc.tile_pool(name="ctxps", bufs=2, space="PSUM")
        )
        pp_pool = top.enter_context(tc.tile_pool(name="pp", bufs=2, space="PSUM"))

        # persistent SBUF tiles
        qT = [qk_pool.tile([P, S], DT, tag="qk", name=f"qT{i}") for i in range(2)]
        kT = [qk_pool.tile([P, S], DT, tag="qk", name=f"kT{i}") for i in range(2)]
        v1 = [
            v_pool.tile([P, HPC * (HD + 1)], DT, tag="v1", name=f"v1_{i}")
            for i in range(NT)
        ]
        ctxT = [
            ctx_pool.tile([P, S], DT, tag="ctxT", name=f"ctxT{i}")
            for i in range(2)
        ]
        wo_sb = [wo_pool.tile([P, D], DT, tag="wo", name=f"wo{i}") for i in range(2)]
        tri_sb = const_pool.tile([P, P], DT, tag="tri")

        w_all = {
            n: w_pool.tile([P, KD * WCOLS], DT, tag="w", name=f"w_{n}")
            for n in ("q", "k", "v")
        }
        wq_sb = [w_all["q"][:, kt * WCOLS : (kt + 1) * WCOLS] for kt in range(KD)]
        wk_sb = [w_all["k"][:, kt * WCOLS : (kt + 1) * WCOLS] for kt in range(KD)]
        wv_sb = [w_all["v"][:, kt * WCOLS : (kt + 1) * WCOLS] for kt in range(KD)]
        xc_all = [
            x_pool.tile([P, KD * 512], DT, tag="xc", name=f"xca{nb}")
            for nb in range(NB)
        ]
        xc = {
            (kt, nb): xc_all[nb][:, kt * 512 : (kt + 1) * 512]
            for kt in range(KD)
            for nb in range(NB)
        }

        # DMA: one batched strided transfer per weight matrix / x block —
        # queue-instruction count (not bandwidth) gates the prologue.
        # Weights on the SP queue, x blocks on the ACT queue.
        for n, src in (("q", wq), ("k", wk), ("v", wv)):
            nc.sync.dma_start(w_all[n][:], src[:])
        nc.sync.dma_start(tri_sb[:], tri[:])
        for nb in range(NB):
            nc.scalar.dma_start(
                xc_all[nb][:], xT[:, nb * KD * 512 : (nb + 1) * KD * 512]
            )
        for i in range(2):
            nc.sync.dma_start(wo_sb[i][:], wo[i * P : (i + 1) * P, :])

        # ones column of v1: memset whole tile once, value region is
        # overwritten by the v-projection evacuations afterwards
        for tt in range(NT):
            nc.vector.memset(v1[tt][:], 1.0)

        # ---- qkv projection fills (one closure per PSUM fill) --------------
        def proj_fills(nb):
            fills = []

            def qk_fill(w_sb, dstT, p, nb=nb):
                def go():
                    ps = pp_pool.tile([P, 512], F32, tag="pp")
                    for kt in range(KD):
                        nc.tensor.matmul(
                            ps[:],
                            w_sb[kt][:, p * P : (p + 1) * P],
                            xc[(kt, nb)][:],
                            start=(kt == 0),
                            stop=(kt == KD - 1),
                        )
                    nc.vector.tensor_copy(
                        dstT[p][:, nb * 512 : (nb + 1) * 512], ps[:]
                    )

                return go

            def v_fill(tloc, nb=nb):
                def go():
                    tt = nb * 4 + tloc
                    ps = pp_pool.tile([P, 512], F32, tag="pp")
                    for kt in range(KD):
                        nc.tensor.matmul(
                            ps[:, 0:WCOLS],
                            xc[(kt, nb)][:, tloc * P : (tloc + 1) * P],
                            wv_sb[kt][:],
                            start=(kt == 0),
                            stop=(kt == KD - 1),
                        )
                    v1_view = v1[tt][:].rearrange("p (a c) -> p a c", c=HD + 1)
                    nc.vector.tensor_copy(
                        v1_view[:, :, 0:HD],
                        ps[:, 0:WCOLS].rearrange("p (a c) -> p a c", c=HD),
                    )

                return go

            # pair-0 work first so attention(nb) can start after 6 fills
            fills.append(qk_fill(wq_sb, qT, 0))
            fills.append(qk_fill(wk_sb, kT, 0))
            for tloc in range(4):
                fills.append(v_fill(tloc))
            fills.append(qk_fill(wq_sb, qT, 1))
            fills.append(qk_fill(wk_sb, kT, 1))
            return fills

        # ---- out-projection fills for query block qb -----------------------
        def outproj_fills(qb, final=False):
            fills = []

            def o_fill(tt, i):
                def go():
                    if final:
                        pso = sc_pool.tile([P, 1024], F32, tag="sc")
                        osb = out_pool.tile([P, 1024], DT, tag="osb")
                        # per-half: 2 mms, then evac (scalar/vector alternate)
                        # and DMA immediately so the tail pipelines
                        for ob in range(2):
                            for kt2 in range(2):
                                nc.tensor.matmul(
                                    pso[:, ob * 512 : ob * 512 + 512],
                                    ctxT[kt2][:, tt * P : (tt + 1) * P],
                                    wo_sb[kt2][:, ob * 512 : (ob + 1) * 512],
                                    start=(kt2 == 0),
                                    stop=(kt2 == 1),
                                )
                            half = slice(ob * 512, ob * 512 + 512)
                            if ob == 0:
                                nc.scalar.copy(osb[:, half], pso[:, half])
                                nc.scalar.dma_start(
                                    out[tt * P : (tt + 1) * P, half], osb[:, half]
                                )
                            else:
                                nc.vector.tensor_copy(osb[:, half], pso[:, half])
                                nc.sync.dma_start(
                                    out[tt * P : (tt + 1) * P, half], osb[:, half]
                                )
                    else:
                        osb = out_pool.tile([P, 1024], DT, tag="osb")
                        for ob in range(2):
                            pso = pp_pool.tile([P, 512], F32, tag="pp")
                            for kt2 in range(2):
                                nc.tensor.matmul(
                                    pso[:],
                                    ctxT[kt2][:, tt * P : (tt + 1) * P],
                                    wo_sb[kt2][:, ob * 512 : (ob + 1) * 512],
                                    start=(kt2 == 0),
                                    stop=(kt2 == 1),
                                )
                            nc.vector.tensor_copy(
                                osb[:, ob * 512 : ob * 512 + 512], pso[:]
                            )
                        nc.sync.dma_start(out[tt * P : (tt + 1) * P, :], osb[:])

                return go

            tlocs = [3] if final else range(4)
            for i, tloc in enumerate(tlocs):
                fills.append(o_fill(qb * 4 + tloc, i))
            return fills

        # ---- attention for query block qb, with interleaved fills ----------
        def emit_attention(qb, fills):
            q0 = qb * 512
            njt = 4 * qb + 4
            nchunks = 2 * ((njt + 1) // 2)
            chunk_i = 0
            for pair in range(2):
                p = pair
                ctxA = ctxps_pool.tile([65, 512], F32, tag="ctxps")
                ctxB = ctxps_pool.tile([65, 512], F32, tag="ctxps")
                for jt0 in range(0, njt, 2):
                    jts = [jt0] if jt0 + 1 >= njt else [jt0, jt0 + 1]
                    ets = []
                    # scores for the chunk (row-group pairs, 64x128 PE mode)
                    for jt in jts:
                        m = jt - 4 * qb
                        lo = P * m if m > 0 else 0
                        scp = sc_pool.tile([P, 1024], F32, tag="sc")
                        for off in (0, 64):
                            nc.tensor.matmul(
                                scp[:, (off // 64) * 512 + lo : (off // 64) * 512 + 512],
                                kT[p][off : off + 64, jt * P : (jt + 1) * P],
                                qT[p][off : off + 64, q0 + lo : q0 + 512],
                                start=True,
                                stop=True,
                            )
                        et = exp_pool.tile([P, 1024], DT, tag="expT")
                        # one exp over both heads, skipping the masked hole
                        scv = scp[:].rearrange("p (two n) -> p two n", two=2)
                        etv = et[:].rearrange("p (two n) -> p two n", two=2)
                        nc.scalar.activation(
                            etv[:, :, lo:512], scv[:, :, lo:512], EXP
                        )
                        if m >= 0:  # diagonal block: triangle mask multiply
                            for half in range(2):
                                h0 = half * 512
                                nc.vector.tensor_mul(
                                    et[:, h0 + lo : h0 + lo + P],
                                    et[:, h0 + lo : h0 + lo + P],
                                    tri_sb[:],
                                )
                        ets.append((et, lo, jt))
                    # interleaved fill work keeps the PE busy during exp
                    chunk_i += 1
                    take = 0
                    if fills:
                        left = nchunks - chunk_i + 1
                        take = max(1, -(-len(fills) // max(1, left)))
                    for _ in range(min(take, len(fills))):
                        fills.pop(0)()
                    # attn @ v for the chunk (128x128 PE mode)
                    for et, lo, jt in ets:
                        for half, ctx in ((0, ctxA), (1, ctxB)):
                            h = 2 * pair + half
                            nc.tensor.matmul(
                                ctx[:, lo:512],
                                v1[jt][:, h * 65 : (h + 1) * 65],
                                et[:, half * 512 + lo : half * 512 + 512],
                                start=(jt == 0),
                                stop=(jt == njt - 1),
                                skip_group_check=True,
                            )
                # normalize: copy denominators to SBUF, reciprocal, broadcast
                # once per pair, scale into ctxT
                rec = small_pool.tile([1, 1024], F32, tag="rec")
                nc.vector.tensor_copy(rec[:, 0:512], ctxA[64:65, :])
                nc.vector.tensor_copy(rec[:, 512:1024], ctxB[64:65, :])
                reci = small_pool.tile([1, 1024], F32, tag="reci")
                nc.vector.reciprocal_approx_fast(reci[:], rec[:])
                recb = small_pool.tile([64, 1024], F32, tag="recb")
                nc.gpsimd.partition_broadcast(recb[:], reci[:], channels=64)
                for half, ctx in ((0, ctxA), (1, ctxB)):
                    nc.vector.tensor_mul(
                        ctxT[p][64 * half : 64 * half + 64, q0 : q0 + 512],
                        ctx[0:64, :],
                        recb[:, half * 512 : half * 512 + 512],
                    )

        # ===== schedule ======================================================
        fills0 = proj_fills(0)
        for f in fills0[:6]:  # pair-0 proj of block 0 inline
            f()
        carry = fills0[6:]
        for qb in range(NB):
            fills = carry
            carry = []
            if qb + 1 < NB:
                fills += proj_fills(qb + 1)
            if qb == 2:
                fills += outproj_fills(0)
            if qb == 3:
                fills += outproj_fills(1) + outproj_fills(2)
            emit_attention(qb, fills)
            for f in fills:  # anything not consumed by the chunk slots
                f()
        # final out-projection. The kt2=0 half only needs pair-0's ctx
        # (ready before pair-1's normalize chain finishes), so pre-issue it
        # into held PSUM slots to keep the PE busy during that chain; then
        # finish kt2=1 + evac + DMA per token tile.
        held = []
        for tt in (12, 13):
            pso = sc_pool.tile([P, 1024], F32, tag="sc")
            for ob in range(2):
                nc.tensor.matmul(
                    pso[:, ob * 512 : ob * 512 + 512],
                    ctxT[0][:, tt * P : (tt + 1) * P],
                    wo_sb[0][:, ob * 512 : (ob + 1) * 512],
                    start=True,
                    stop=False,
                    skip_group_check=True,
                )
            held.append((tt, pso, 1024))
        for ob in range(2):
            pp = pp_pool.tile([P, 512], F32, tag="pp")
            nc.tensor.matmul(
                pp[:],
                ctxT[0][:, 14 * P : 15 * P],
                wo_sb[0][:, ob * 512 : (ob + 1) * 512],
                start=True,
                stop=False,
                skip_group_check=True,
            )
            held.append((14, pp, ob))
        for tt, pso, kind in held:
            if kind == 1024:
                osb = out_pool.tile([P, 1024], DT, tag="osb")
                for ob in range(2):
                    nc.tensor.matmul(
                        pso[:, ob * 512 : ob * 512 + 512],
                        ctxT[1][:, tt * P : (tt + 1) * P],
                        wo_sb[1][:, ob * 512 : (ob + 1) * 512],
                        start=False,
                        stop=True,
                        skip_group_check=True,
                    )
                    half = slice(ob * 512, ob * 512 + 512)
                    if ob == 0:
                        nc.scalar.copy(osb[:, half], pso[:, half])
                        nc.scalar.dma_start(
                            out[tt * P : (tt + 1) * P, half], osb[:, half]
                        )
                    else:
                        nc.vector.tensor_copy(osb[:, half], pso[:, half])
                        nc.sync.dma_start(
                            out[tt * P : (tt + 1) * P, half], osb[:, half]
                        )
            else:
                ob = kind
                half = slice(ob * 512, ob * 512 + 512)
                nc.tensor.matmul(
                    pso[:],
                    ctxT[1][:, tt * P : (tt + 1) * P],
                    wo_sb[1][:, ob * 512 : (ob + 1) * 512],
                    start=False,
                    stop=True,
                    skip_group_check=True,
                )
                osb = out_pool.tile([P, 512], DT, tag="osbh")
                if ob == 0:
                    nc.scalar.copy(osb[:], pso[:])
                    nc.scalar.dma_start(out[tt * P : (tt + 1) * P, half], osb[:])
                else:
                    nc.vector.tensor_copy(osb[:], pso[:])
                    nc.sync.dma_start(out[tt * P : (tt + 1) * P, half], osb[:])
        for f in outproj_fills(NB - 1, final=True):
            f()


_BUILD_CACHE = {}


def build():
    if "nc" in _BUILD_CACHE:
        return _BUILD_CACHE["nc"]
    nc = bacc.Bacc("TRN2", target_bir_lowering=False, debug=False)
    aps = {
        "xT": nc.dram_tensor("xT", [P, NB * KD * 512], DT, kind="ExternalInput").ap(),
        "wq": nc.dram_tensor("wq", [P, KD * WCOLS], DT, kind="ExternalInput").ap(),
        "wk": nc.dram_tensor("wk", [P, KD * WCOLS], DT, kind="ExternalInput").ap(),
        "wv": nc.dram_tensor("wv", [P, KD * WCOLS], DT, kind="ExternalInput").ap(),
        "wo": nc.dram_tensor("wo", [WCOLS, D], DT, kind="ExternalInput").ap(),
        "tri": nc.dram_tensor("tri", [P, P], DT, kind="ExternalInput").ap(),
        "out": nc.dram_tensor("out", [S, D], DT, kind="ExternalOutput").ap(),
    }
    with tile.TileContext(nc) as tc:
        _emit(tc, aps)
    nc.compile()
    _BUILD_CACHE["nc"] = nc
    return nc


def make_tri() -> np.ndarray:
    """tri[dj, t] = 1 if dj <= t else 0 (causal keep within a 128 block)."""
    dj = np.arange(P)[:, None]
    t = np.arange(P)[None, :]
    return prep(np.where(dj <= t, 1.0, 0.0).astype(np.float32))


def swz_w(w):
    """[D, WCOLS] -> [P, KD*WCOLS]: SBUF-layout swizzle so the DMA is contiguous."""
    return np.ascontiguousarray(
        w.reshape(KD, P, WCOLS).transpose(1, 0, 2).reshape(P, KD * WCOLS)
    )


def swz_x(xb):
    """x[b] [S, D] -> xT swizzled [P, NB*KD*512] matching xc_all layout."""
    xT = xb.T  # [D, S]
    return np.ascontiguousarray(
        xT.reshape(KD, P, NB, 512).transpose(1, 2, 0, 3).reshape(P, NB * KD * 512)
    )


def make_in_maps(x, w_qkv, w_out):
    tri = make_tri()
    scale = 1.0 / np.sqrt(HD)
    in_maps = []
    for c in range(NCORES):
        b, g = c // 4, c % 4
        cs = slice(g * WCOLS, (g + 1) * WCOLS)
        in_maps.append(
            {
                "xT": prep(swz_x(x[b])),
                "wq": prep(swz_w(w_qkv[:, g * WCOLS : (g + 1) * WCOLS] * scale)),
                "wk": prep(swz_w(w_qkv[:, D + g * WCOLS : D + (g + 1) * WCOLS])),
                "wv": prep(swz_w(w_qkv[:, 2 * D + g * WCOLS : 2 * D + (g + 1) * WCOLS])),
                "wo": prep(w_out[cs, :]),
                "tri": tri,
            }
        )
    return in_maps


def kernel(x, w_qkv, w_out, _trace=False):
    nc = build()
    in_maps = make_in_maps(
        np.asarray(x, np.float32), np.asarray(w_qkv, np.float32),
        np.asarray(w_out, np.float32),
    )
    res = bass_utils.run_bass_kernel_spmd(
        nc, in_maps, core_ids=list(range(NCORES)), trace=_trace
    )
    outs = [np.asarray(res.results[c]["out"], np.float32) for c in range(NCORES)]
    full = np.stack(
        [sum(outs[b * 4 : (b + 1) * 4][1:], outs[b * 4]) for b in range(B)], axis=0
    )
    if _trace:
        kernel.last_results = res
    return full.astype(np.float32)
